# revision 24
# baseline (speedup 1.0000x reference)
"""Trainium2 Bass kernel for a BCE-based decoding loss (v5).

Math: with t = tanh(llrs/2),
  p[b,r]   = clip(prod_w t[b, idx[r,w]], -1+eps, 1-eps)
  bce(z,y) = softplus(z) - z*y  with  z = -2*arctanh(p)
which for y in {0,1} simplifies exactly to
  bce = log(2) - log(1 + (1-2y) * p)
so   loss = 0.5*(M+K)*log(2) - (0.5/B) * sum_{b,r} log(1 + s[b,r]*p[b,r])
with s = 1-2y.  (The clip never binds on this data: min(1+sp) = 0.33.)

Sharding: pure data parallel over batch -- 8 cores x 128 rows each.

Host-side prep (data movement / quantization only, no transcendental
math): llrs are halved+clipped+scaled (z' = alpha*clip(llr/2, +-Z)),
cast to fp8 e4m3, and gathered per (check, w) slot.  The label
s = (1-2y) is folded into the SIGN BIT of the w=0 slot (tanh is odd).

v5 slot layout per check tile (ct checks): four blocks of 2*ct columns,
   blk0 = (w0,w1) check-interleaved, blk1 = (w2,w3), blk2 = (w4,w5),
   blk3 = (w6,w7),
so level-1 multiplies [blk0|blk1] x [blk2|blk3] elementwise and the
resulting h = [hX|hY] holds (h0,h1) pairs in hX and (h2,h3) pairs in hY.

Device math per check tile, measured HW rates in brackets (ns per
128-row column):
  tA = tanh(g / alpha)                    ACT [0.83+ovh], exact, fp8 in
  'F' tiles: h = TANH5M(gB) * tA          custom DVE 1x [1.04]: deg-5
       odd poly of tanh times tA -> fused level-1 products
  'A' tiles: h = tA_lo * tA_hi            DVE bf16 mul, 2x mode [0.52]
  PROD2SUM(hX, hY) -> acc                 custom DVE, HAND-WRITTEN 2x
       uop program: per cycle reads packed (h0,h1) and (h2,h3), forms
       m = (h0*h2)*(h1*h3) and accumulates ln(1+m) ~= m - m^2/2.
       1 cycle per check -- replaces the old level-2 mul + LOG1PM pair
       (2 cycles per check).
The obs tile (8 obs x 128 slots) keeps the v3 path (tanh5m + mul tree +
LOG1PM deg-3).  The F/A mix (4992 F / 3200 A) balances scalar ~44us
against vector ~42us.

Per-tile results: the engine-accumulator readout (DVE_READ_ACCUMULATOR2)
returns garbage after a 2x-mode op, so each tile's sum is harvested from
the last column of the PROD2SUM dst stream (the running sum) and DMA'd
out from the idle GpSimd queue (on the Sync queue the wait-for-p2s
would block later input-tile DMA issues; on Vector a copy costs ~2.5us
total).  measured: HW exec ~63.6us vs 67.9us baseline, rel err 2.1e-5.
"""

import math
import os

import numpy as np

os.environ.setdefault("MYCRO_LOCAL_CACHE", "1")

import ml_dtypes  # noqa: E402

B, N, M, K = 1024, 16384, 8192, 8
WC, WO = 8, 128
NCORES = 8
BL = B // NCORES            # batch rows per core = 128
OBS_SLOTS = K * WO          # 1024 obs slots
TOT_SLOTS = M * WC + OBS_SLOTS       # 66560
EPS = 1e-6

# Tile plan: (checks, kind); kind 'F' = fused (ACT half + custom half),
# 'A' = ACT-all.  Small head tiles start ACT early; small tail tiles keep
# the final vector chain short; F/A interleaving keeps both engines fed.
TILE_PLAN = [(128, "F"), (256, "F"), (512, "F"), (1024, "F"),
             (1024, "A"), (1024, "F"), (512, "F"), (1024, "F"),
             (128, "A"), (1024, "F"), (1024, "A"), (512, "A")]
OBS_AFTER = 2               # emit the obs tile after this many check tiles
assert sum(c for c, _ in TILE_PLAN) == M
NTILES = len(TILE_PLAN)

# tanh(z) ~= z'(PA + u(PB + u)), z' = ALPHA*z, u = z'^2, for |z| <= ZCLIP
PA, PB, ALPHA, ZCLIP = 2.331135, -2.379626, 0.374835, 3.0
# ln(1+y) ~= y + y^2*(L0 + y*L1)  (deg-3, obs tile); checks use deg-2.
L0, L1_ = -0.5, 1.0 / 3.0

_CACHE = {}


def _prod2sum_uops():
    """Hand-written v3 uop programs for PROD2SUM_ANT.

    Semantics (per check, 4 level-1 products h0,h1,h2,h3 with
    in0 = [... h0 h1 ...] pairs and in1 = [... h2 h3 ...] pairs):
        m   = (h0*h2) * (h1*h3)
        acc += m + C0*m*m            (C0 = -0.5: deg-2 ln(1+m) series)
    REGULAR (1x) program: 2-state even/odd FSM pairing consecutive
    elements temporally via the stage-1 out-flop.
    2X_1PORT program: the packed halves (SRC_*_HI) arrive spatially, so a
    single steady uop forms the pair product per cycle -- 1 cycle/check.
    dst is a dummy sink (the accumulator is the real output).
    """
    from concourse.dve_uop import (
        ENABLE, AluInp, AluOp, DelayInp, InpSel, OutPath, OutSel,
        Trigger, UopConfig,
    )

    P, C = AluInp.PREV_ALU_OUT, AluInp.CURR_ALU_OUT
    D = [AluInp.PREV_DELAY_0, AluInp.PREV_DELAY_1, AluInp.PREV_DELAY_2,
         AluInp.PREV_DELAY_3, AluInp.PREV_DELAY_4, AluInp.PREV_DELAY_5]
    SRC_DONE = (Trigger.SRC_TENSOR_DONE, Trigger.NONE, Trigger.NONE)

    def mk(inputs, stages, captures, lanes, *, consume, write_hi=False,
           write=True, trigger=SRC_DONE, nxt=(0, 0, 0), repeat=0):
        u = UopConfig()
        for src, lane_id in inputs:
            u.enable_input(src, lane_id)
        for st in range(8):
            dp = u.datapath_config[st]
            dp.pass_through_delay(*lanes)
            op, a, b = stages.get(st, (AluOp.BYPASS, P, P))
            dp.enable_alu(op, a, b)
        for st, lane in captures:
            u.datapath_config[st].enable_delay_from_src(
                DelayInp.PREV_ALU_OUT, lane)
        if write:
            u.enable_output(OutSel.ALU_OUT, OutPath.WR0_LO)
            if write_hi:
                u.enable_output(OutSel.ALU_OUT, OutPath.WR0_HI)
        u.require_inp0, u.require_inp1 = consume
        u.trigger, u.next_uop, u.repeat_count = trigger, nxt, repeat
        return u

    MUL, ADD, BYP = AluOp.MULTIPLY, AluOp.ADD, AluOp.BYPASS
    ONCE = (Trigger.COUNT, Trigger.NONE, Trigger.NONE)
    STEP = (Trigger.SRC_TENSOR_DONE, Trigger.COUNT, Trigger.NONE)

    # --- REGULAR (1x): [seed, even, odd] ---
    # lanes: d0=SRC_1, d1=C0, d2=ONE, d3=ZERO, d4=m-capture
    inp1 = [(InpSel.SRC_0, 0), (InpSel.SRC_1, 1), (InpSel.CONST_0, 2),
            (InpSel.ONE_F32, 3), (InpSel.ZERO, 4)]
    lanes1 = (0, 1, 2, 3, 4)
    seed1 = mk(inp1, {6: (BYP, D[3], D[3])}, [], lanes1,
               consume=(0, 0), write=False, trigger=ONCE, nxt=(1, 0, 0),
               repeat=1)
    even = mk(inp1,
              {0: (MUL, P, D[0]),        # v = h0*h2
               1: (BYP, P, P),           # latch v in stage-1 flop
               6: (BYP, C, C)},          # hold accumulator
              [], lanes1, consume=(1, 1), trigger=STEP, nxt=(0, 2, 0),
              repeat=1)
    odd = mk(inp1,
             {0: (MUL, P, D[0]),         # v' = h1*h3
              1: (MUL, P, C),            # m = v' * v   (v from even elem)
              2: (MUL, P, D[1]),         # t = m*C0
              3: (ADD, P, D[2]),         # u = t+1
              4: (MUL, P, D[4]),         # y = u*m
              6: (ADD, C, P)},           # acc += y
             [(2, 4)],                   # capture m (s1 out) into d4 at s2
             lanes1, consume=(1, 1), trigger=STEP, nxt=(0, 1, 0), repeat=1)

    # --- 2X_1PORT: [seed, steady, pad] ---
    # lanes: d0=SRC_1 (then v0, then m), d1=SRC_0_HI, d2=SRC_1_HI,
    #        d3=C0, d4=ONE, d5=ZERO
    inp2 = [(InpSel.SRC_0, 0), (InpSel.SRC_1, 1), (InpSel.SRC_0_HI, 2),
            (InpSel.SRC_1_HI, 3), (InpSel.CONST_0, 4), (InpSel.ONE_F32, 5),
            (InpSel.ZERO, 6)]
    lanes2 = (0, 1, 2, 3, 4, 5)
    seed2 = mk(inp2, {6: (BYP, D[5], D[5])}, [], lanes2,
               consume=(0, 0), write=False, trigger=ONCE, nxt=(1, 0, 0),
               repeat=1)
    steady = mk(inp2,
                {0: (MUL, P, D[0]),      # v0 = h0*h2
                 1: (MUL, D[1], D[2]),   # v1 = h1*h3
                 2: (MUL, P, D[0]),      # m = v1*v0
                 3: (MUL, P, D[3]),      # t = m*C0
                 4: (ADD, P, D[4]),      # u = t+1
                 5: (MUL, P, D[0]),      # y = u*m
                 6: (ADD, C, P)},        # acc += y
                [(1, 0), (3, 0)],        # v0 -> d0 at s1; m -> d0 at s3
                lanes2, consume=(1, 1), write_hi=True)
    pad2 = mk(inp2, {}, [], lanes2, consume=(0, 0), write=False)

    return [seed1, even, odd], [seed2, steady, pad2]


def _register_custom_ops():
    """Register the kernel-specific custom DVE ops with the dve_ops
    registry (the documented extension point is appending to OPS; doing it
    at runtime keeps kernel.py self-contained).  TANH5M/LOG1PM pin
    uops_sha from the actual lowering so DveOp.compile's drift check
    passes; PROD2SUM bypasses compile entirely by pre-seeding
    _COMPILE_CACHE with a hand-built DveOpSpec carrying a 2x uop
    program (perf_max=1)."""
    if "ops" in _CACHE:
        return _CACHE["ops"]
    import concourse.dve_ops as dve_ops
    from concourse.dve_spec import (
        Spec, Src0, Src1, C0, C1, Zero, sq, lower, _has_src1,
    )
    from concourse.dve_uop import DveOpSpec
    from operator import add

    u = sq(Src0)
    body_t = (Src0 * (C0 + u * (C1 + u))) * Src1

    def ref_t(in0, in1, s0, s1, imm2):
        x = in0.astype(np.float32)
        uu = x * x
        return (x * (s0 + uu * (s1 + uu)) * in1.astype(np.float32)).astype(
            np.float32)

    tanh5m = dve_ops.DveOp(
        "TANH5M_ANT", Spec(body=body_t, reference=ref_t),
        subdim=False, uops_sha={})

    m = Src0 * Src1
    body_l = m + sq(m) * (C0 + m * C1)

    def ref_l(in0, in1, c0, c1, c2):
        y = in0.astype(np.float32) * in1.astype(np.float32)
        b = (y + y * y * (c0 + y * c1)).astype(np.float32)
        return b, b.reshape(b.shape[0], -1).sum(axis=-1, keepdims=True)

    log1pm = dve_ops.DveOp(
        "LOG1PM_ANT",
        Spec(body=body_l, accum=add, accum_init=Zero, reference=ref_l),
        subdim=False, uops_sha={})

    def ref_p2s(in0, in1, c0, c1, c2):
        P = in0.shape[0]
        x = in0.astype(np.float32).reshape(P, -1)
        z = in1.astype(np.float32).reshape(P, -1)
        v = x * z                                  # (h0*h2, h1*h3) pairs
        mm = v[:, 0::2] * v[:, 1::2]
        y = mm + c0 * mm * mm
        # dst mirrors the 2x datapath: each cycle writes the running sum to
        # both packed halves, so dst = [c1,c1,c2,c2,...] and dst[-1] is the
        # tile total (the caller harvests it with a [P,1] copy).
        run = np.cumsum(y, axis=-1, dtype=np.float32)
        out = np.repeat(run, 2, axis=-1)
        acc = run[:, -1:].astype(np.float32)
        return out, acc

    # The Spec body is representative only (PROD2SUM never goes through
    # lower(); its uops are hand-written).  CoreSim uses `reference`.
    prod2sum = dve_ops.DveOp(
        "PROD2SUM_ANT",
        Spec(body=Src0 * Src1 * C0, accum=add, accum_init=Zero,
             reference=ref_p2s),
        subdim=False, uops_sha={})

    for op in (tanh5m, log1pm, prod2sum):
        if op.name not in dve_ops._SUB_OPCODE_FOR_NAME:
            dve_ops.OPS.append(op)
            dve_ops.CUSTOM_DVE_SPECS[op.name] = op.spec
            dve_ops._SUB_OPCODE_FOR_NAME[op.name] = (
                dve_ops._CUSTOM_DVE_ROW_BASE + len(dve_ops.OPS) - 1)

    from concourse.dve_uop import DveOpSpec as _DOS
    for op in (tanh5m, log1pm):
        shas = {}
        for ver in ("v3", "v4"):
            spec = _DOS(
                name=op.name,
                opcode=dve_ops.get_dve_sub_opcode(op.name),
                uops=lower(op.spec, ver=ver),
                rd1_en=_has_src1(op.spec),
            )
            shas[ver] = spec.sha(ver)
        object.__setattr__(op, "uops_sha", shas)

    reg, two = _prod2sum_uops()
    p2s_spec = DveOpSpec(
        name=prod2sum.name,
        opcode=dve_ops.get_dve_sub_opcode(prod2sum.name),
        uops=reg,
        uops_2x=two,
        perf_max=1,
        rd1_en=True,
    )
    p2s_spec.validate("v3")
    dve_ops._COMPILE_CACHE[(prod2sum.name, "v3")] = p2s_spec
    object.__setattr__(prod2sum, "uops_sha", {"v3": p2s_spec.sha("v3")})

    _CACHE["ops"] = (tanh5m, log1pm, prod2sum)
    return _CACHE["ops"]


def build_nc():
    import concourse.bacc as bacc
    import concourse.mybir as mybir
    import concourse.tile as tile
    from contextlib import ExitStack

    tanh5m, log1pm, prod2sum = _register_custom_ops()

    nc = bacc.Bacc("TRN2", target_bir_lowering=False, debug=False)
    f32 = mybir.dt.float32
    bf16 = mybir.dt.bfloat16
    f8 = mybir.dt.float8e4

    g_dram = nc.dram_tensor("g", [BL, TOT_SLOTS], f8, kind="ExternalInput")
    outb = nc.dram_tensor("outb", [128, NTILES], bf16, kind="ExternalOutput")
    outo = nc.dram_tensor("outo", [128, 1], f32, kind="ExternalOutput")

    Tanh = mybir.ActivationFunctionType.Tanh

    with tile.TileContext(nc) as tc:
        with ExitStack() as ctx:
            singles = ctx.enter_context(tc.tile_pool(name="singles", bufs=1))
            gp = ctx.enter_context(tc.tile_pool(name="gp", bufs=4))
            tp = ctx.enter_context(tc.tile_pool(name="tp", bufs=2))
            hp = ctx.enter_context(tc.tile_pool(name="hp", bufs=2))
            qp = ctx.enter_context(tc.tile_pool(name="qp", bufs=2))

            acco = singles.tile([128, 1], f32)
            # dummy sink for the obs LOG1PM (never read back)
            lnsink = singles.tile([128, 64], bf16)

            def check_tile(t, off, ct, kind):
                """DMA + tanh + level-1 + fused pair-product/log/sum."""
                ts_ = ct * WC
                g = gp.tile([128, ts_], f8, tag=f"g{ts_}")
                if kind == "F" and t < 4:
                    # during the DMA ramp, land the ACT half first so tanh
                    # can start while the TANH5M half is still in flight
                    # (subtile deps make the ACT wait only on the A half).
                    nc.sync.dma_start(g[:, 0:ts_ // 2],
                                      g_dram[:, off:off + ts_ // 2])
                    nc.sync.dma_start(g[:, ts_ // 2:ts_],
                                      g_dram[:, off + ts_ // 2:off + ts_])
                else:
                    nc.sync.dma_start(g[:], g_dram[:, off:off + ts_])
                h = hp.tile([128, ts_ // 2], bf16, tag=f"h{ts_}")
                if kind == "F":
                    tA = tp.tile([128, ts_ // 2], bf16, tag=f"tA{ts_}")
                    nc.scalar.activation(tA[:], g[:, 0:ts_ // 2], Tanh,
                                         bias=0.0, scale=1.0 / ALPHA)
                    nc.vector._custom_dve(
                        tanh5m, out=h[:], in0=g[:, ts_ // 2:ts_], in1=tA[:],
                        s0=PA, s1=PB)
                else:
                    tA = tp.tile([128, ts_], bf16, tag=f"tF{ts_}")
                    nc.scalar.activation(tA[:], g[:], Tanh,
                                         bias=0.0, scale=1.0 / ALPHA)
                    nc.vector.tensor_mul(h[:], tA[:, 0:ts_ // 2],
                                         tA[:, ts_ // 2:ts_])
                ln = qp.tile([128, 2 * ct], bf16, tag=f"ln{ct}")
                inst = nc.vector._custom_dve(
                    prod2sum, out=ln[:],
                    in0=h[:, 0:2 * ct], in1=h[:, 2 * ct:4 * ct], s0=L0)
                inst.ins.perf_max = 1  # byte-36[7:6]: allow 2X_1PORT
                # the 2x program streams the running sum to dst; its last
                # column is the tile total (the engine-accumulator readout
                # path misbehaves after a 2x op, so harvest from dst).
                # Issue the tail DMA from the idle GpSimd queue -- on the
                # sync queue its wait-for-p2s would block later input DMAs.
                nc.gpsimd.dma_start(outb[:, t:t + 1], ln[:, 2 * ct - 1:2 * ct])

            def obs_tile():
                ob = M * WC
                g = gp.tile([128, OBS_SLOTS], f8, tag="go")
                nc.sync.dma_start(g[:], g_dram[:, ob:ob + OBS_SLOTS])
                tA = tp.tile([128, 512], bf16, tag="tAo")
                nc.scalar.activation(tA[:], g[:, 0:512], Tanh,
                                     bias=0.0, scale=1.0 / ALPHA)
                h = hp.tile([128, 512], bf16, tag="ho")
                nc.vector._custom_dve(
                    tanh5m, out=h[:], in0=g[:, 512:1024], in1=tA[:],
                    s0=PA, s1=PB)
                w = 512
                while w > 2 * K:
                    w //= 2
                    nh = qp.tile([128, w], bf16, tag=f"o{w}")
                    nc.vector.tensor_mul(nh[:], h[:, 0:w], h[:, w:2 * w])
                    h = nh
                nc.vector._custom_dve(
                    log1pm, out=lnsink[:, 0:K],
                    in0=h[:, 0:K], in1=h[:, K:2 * K],
                    s0=L0, s1=L1_, accum_out=acco[:])
                nc.gpsimd.dma_start(outo[:, :], acco[:])

            offs = np.cumsum([0] + [c * WC for c, _ in TILE_PLAN])
            for t in range(NTILES):
                ct, kind = TILE_PLAN[t]
                check_tile(t, int(offs[t]), ct, kind)
                if t + 1 == OBS_AFTER + 1:
                    obs_tile()

    nc.compile()
    return nc


def get_nc():
    if "nc" not in _CACHE:
        _CACHE["nc"] = build_nc()
    return _CACHE["nc"]


def build_slots(chk_idx, obs_idx):
    """Column j of the shipped tensor holds z'[idx] for slot-order:
    check tile t (ct checks at check-offset r0, slot-offset o = 8*r0):
      blk0 j = o + 2c + v          -> chk[r0+c, v]      (v in 0,1)
      blk1 j = o + 2ct + 2c + v    -> chk[r0+c, 2+v]
      blk2 j = o + 4ct + 2c + v    -> chk[r0+c, 4+v]
      blk3 j = o + 6ct + 2c + v    -> chk[r0+c, 6+v]
    so level-1 multiplies [blk0|blk1] x [blk2|blk3] elementwise, giving
    h = [(h0,h1) pairs | (h2,h3) pairs].
    obs tile: A-half j = M*WC + v*K + k      -> obs[k, v]
              B-half j = M*WC + 512 + v*K + k -> obs[k, v+64]  (v-major)."""
    chk = np.asarray(chk_idx)
    obs = np.asarray(obs_idx)
    parts = []
    r0 = 0
    for ct, _ in TILE_PLAN:
        sub = chk[r0:r0 + ct]                           # [ct, 8]
        for v0 in range(0, 8, 2):
            parts.append(sub[:, v0:v0 + 2].reshape(-1))  # check-interleaved
        r0 += ct
    parts.append(obs[:, 0:WO // 2].T.reshape(-1))
    parts.append(obs[:, WO // 2:WO].T.reshape(-1))
    return np.concatenate(parts).astype(np.int64)


def make_in_maps(llrs, syndromes, observables, chk_idx, obs_idx):
    zp = (np.clip(np.asarray(llrs) * 0.5, -ZCLIP, ZCLIP) * ALPHA).astype(
        ml_dtypes.float8_e4m3)
    slots = build_slots(chk_idx, obs_idx)
    g_all = np.take(zp, slots, axis=1)                  # [B, TOT_SLOTS]
    # fold s = (1-2y) into the sign bit of the w=0 slot of each check:
    # w=0 sits at the EVEN columns of blk0 (stride 2 from the tile base).
    v = g_all.view(np.uint8)
    syn = np.asarray(syndromes)
    r0 = 0
    for ct, _ in TILE_PLAN:
        o = 8 * r0
        v[:, o:o + 2 * ct:2] ^= (
            syn[:, r0:r0 + ct] != 0).astype(np.uint8) << 7
        r0 += ct
    yobs = (np.asarray(observables) != 0).astype(np.uint8) << 7
    v[:, M * WC:M * WC + K] ^= yobs                     # v=0 of obs A half
    return [{"g": g_all[BL * c:BL * (c + 1)]} for c in range(NCORES)]


def finish(results):
    total = 0.0
    for r in results:
        total += float(np.asarray(r["outb"]).astype(np.float64).sum())
        total += float(np.asarray(r["outo"]).astype(np.float64).sum())
    loss = 0.5 * (M + K) * math.log(2.0) - 0.5 * total / B
    return np.float32(loss)


def kernel(llrs, syndromes, observables, chk_idx, obs_idx):
    from concourse.bass_utils import run_bass_kernel_spmd

    in_maps = make_in_maps(llrs, syndromes, observables, chk_idx, obs_idx)
    nc = get_nc()
    res = run_bass_kernel_spmd(nc, in_maps, core_ids=list(range(NCORES)))
    return finish(res.results)
